# revision 10
# baseline (speedup 1.0000x reference)
"""Trainium2 Bass kernel for nn_Explainer: out[b] = sum_k w[b,k] * (archs[k] off-diag).

Equivalent to a (2048,32) @ (32,65536) matmul with the diagonal of each
256x256 archetype zeroed. Sharding: the 65536 output columns are split across
the 8 cores (8192 columns each) — each core then only needs a small slice of
the archetypes plus the full (tiny) weight matrix; the per-core output write
is the roofline.

Precision: fp16 end to end on-device (inputs quantized to fp16 on host, PE
matmul fp16 -> fp32 PSUM, output stored fp16, widened to fp32 on host). This
halves the HBM store traffic (64 MB -> 32 MB/core) vs an fp32 kernel — the
DMA-write roofline — and keeps the PE at full 16-bit rate. Normalized error
vs the fp32 reference is ~5e-4, well inside the 2e-2 gate.

Per-core device layout (all host-side prepped so every DMA is a plain copy):
  wt4   (128, 2048): batch_weights^T replicated into 4 row-groups
                     wt4[32a+k, b] = w[b, k]
  archp (128, 2048): the core's 8192 archetype columns, masked, split into
                     16 chunks of 512; chunk t lives in row-group a = t%4 at
                     quad j = t//4: archp[32a+k, 512j+c] = A[k, 512t+c]
  out   (2048, 8192): the core's output column slice, natural order.

Compute: for each 128-row batch tile, 4 quads x 4 row-tiled fp16 matmuls
(K=32 at tile_position (32a,0)), PSUM split into 1024-col tiles (4 bufs =
all 8 banks) drained by alternating Vector/Scalar casts (fp32 PSUM -> fp16
SBUF; the only two PSUM-capable drain engines on TRN2, both ~95% busy —
the compute-side pace, ~77 us, just under the ~80 us DMA-store pace).

The two loads the first matmul needs go out on the gpsimd SWDGE queue,
which finishes its preamble ~2 us before the sync HWDGE queue can issue.
Stores ride the sync HWDGE queue with a graded size ramp (512 KB quads
while the 16-engine DMA pipe fills, 1 MB halves next, then steady-state
2 MB full tiles at the 26.6 GB/s/engine line rate, and 1 MB halves again
for the last batch tile so the final store drains in ~2.4 us). Staging
pools are deep (6/4/5 bufs = ~14 MB) so the copy stream never stalls on
buffer recycling while the DMA works down its ramp-phase backlog — that
stall is what starved the DMA mid-kernel in earlier revisions.
"""

import numpy as np

import concourse.tile as tile
from concourse import bacc, mybir
from concourse.bass_utils import run_bass_kernel_spmd

B, K, D = 2048, 32, 256
NCORES = 8
COLS = D * D            # 65536
CPC = COLS // NCORES    # 8192 columns per core
GW = CPC // 4           # 2048 columns per row-group
MT = 128                # batch tile rows (psum partition dim)
NMT = B // MT           # 16 batch tiles
NQ = GW // 512          # 4 quads per batch tile

F16 = mybir.dt.float16
F32 = mybir.dt.float32

_compiled = {}


def _build():
    nc = bacc.Bacc("TRN2", target_bir_lowering=False, debug=False, num_devices=NCORES)
    wt = nc.dram_tensor("wt4", [128, B], F16, kind="ExternalInput").ap()
    # archp stored chunk-major: chunk j is a fully contiguous 128 KB block,
    # so the per-chunk prologue loads run at DMA line rate.
    ar = nc.dram_tensor("archp", [NQ, 128, 512], F16, kind="ExternalInput").ap()
    out = nc.dram_tensor("out", [B, CPC], F16, kind="ExternalOutput").ap()

    with tile.TileContext(nc) as tc:
        with (
            tc.tile_pool(name="wpool", bufs=1) as wpool,
            tc.tile_pool(name="apool", bufs=1) as apool,
            tc.tile_pool(name="pspool", bufs=4, space="PSUM") as pspool,
            tc.tile_pool(name="stpool", bufs=8) as stpool,
        ):
            # Chunked input loads so the first matmuls (needing only wt4 and
            # archp chunk 0) start a couple of microseconds in rather than
            # waiting for the full 1 MB of inputs. All on the sync HWDGE
            # queue: the gpsimd software-DGE queue pays ~4 us of descriptor
            # generation and the scalar HWDGE queue is no earlier, so the
            # simple ordering wins.
            wt_sb = wpool.tile([128, B], F16)
            ar_sb = apool.tile([128, GW], F16)
            nc.sync.dma_start(wt_sb[:, :MT], wt[:, :MT])
            nc.sync.dma_start(ar_sb[:, :512], ar[0])
            nc.sync.dma_start(ar_sb[:, 512:1024], ar[1])
            nc.sync.dma_start(wt_sb[:, MT:], wt[:, MT:])
            for j in range(2, NQ):
                nc.sync.dma_start(ar_sb[:, 512 * j : 512 * (j + 1)], ar[j])

            cnt = 0
            for m in range(NMT):
                # Store-size ramp: small while the DMA pipe fills, full-tile
                # in steady state, small again for the final tile's drain.
                if m < 2:
                    quads_per_store = 1
                elif m < 4:
                    quads_per_store = 2
                else:
                    quads_per_store = NQ
                for j in range(NQ):
                    if j % quads_per_store == 0:
                        st = stpool.tile(
                            [128, GW * quads_per_store],
                            F16,
                            tag=f"st{quads_per_store}",
                            bufs={1: 4, 2: 2, NQ: 3}[quads_per_store],
                        )
                    for h in range(2):
                        ps = pspool.tile([128, 2 * 512], F32)
                        for i, a in enumerate((2 * h, 2 * h + 1)):
                            nc.tensor.matmul(
                                ps[:, 512 * i : 512 * (i + 1)],
                                wt_sb[32 * a : 32 * (a + 1), MT * m : MT * (m + 1)],
                                ar_sb[32 * a : 32 * (a + 1), 512 * j : 512 * (j + 1)],
                                start=True,
                                stop=True,
                                tile_position=(32 * a, 0),
                            )
                        off = GW * (j % quads_per_store) + 1024 * h
                        dst = st[:, off : off + 1024]
                        # Alternate the PSUM->SBUF drain (fp32 -> fp16 cast)
                        # between the two PSUM-capable engines.
                        if cnt % 2 == 0:
                            nc.vector.tensor_copy(dst, ps[:])
                        else:
                            nc.scalar.copy(dst, ps[:])
                        cnt += 1
                    if (j + 1) % quads_per_store == 0:
                        j0 = j + 1 - quads_per_store
                        nc.sync.dma_start(
                            out[MT * m : MT * (m + 1), GW * j0 : GW * (j + 1)],
                            st[:],
                        )

    nc.compile()
    return nc


def _get_nc():
    if "nc" not in _compiled:
        _compiled["nc"] = _build()
    return _compiled["nc"]


def _prep_inputs(batch_weights: np.ndarray, archs: np.ndarray):
    w = np.asarray(batch_weights, dtype=np.float32)
    A = np.asarray(archs, dtype=np.float32).reshape(K, COLS).copy()
    A[:, :: D + 1] = 0.0  # zero the diagonal of each (D, D) archetype
    A = A.astype(np.float16)

    wt4 = np.ascontiguousarray(np.tile(w.T.astype(np.float16), (4, 1)))  # (128, B)

    in_maps = []
    for c in range(NCORES):
        sl = A[:, CPC * c : CPC * (c + 1)].reshape(K, 4 * NQ, 512)
        archp = np.concatenate(
            [sl[:, a::4, :].reshape(K, NQ, 512) for a in range(4)], axis=0
        )  # (128, NQ, 512); chunk-major DRAM layout is (NQ, 128, 512)
        in_maps.append(
            {"wt4": wt4, "archp": np.ascontiguousarray(archp.transpose(1, 0, 2))}
        )
    return in_maps


def _gather(results) -> np.ndarray:
    outf = np.empty((B, COLS), dtype=np.float32)
    for c in range(NCORES):
        outf[:, CPC * c : CPC * (c + 1)] = results[c]["out"]
    return outf.reshape(B, D, D)


def kernel(batch_weights: np.ndarray, archs: np.ndarray, **run_kwargs) -> np.ndarray:
    nc = _get_nc()
    in_maps = _prep_inputs(batch_weights, archs)
    res = run_bass_kernel_spmd(nc, in_maps, list(range(NCORES)), **run_kwargs)
    if run_kwargs:
        _compiled["last_result"] = res
    return _gather(res.results)


# revision 13
# speedup vs baseline: 1.0164x; 1.0164x over previous
"""Trainium2 Bass kernel for nn_Explainer: out[b] = sum_k w[b,k] * (archs[k] off-diag).

Equivalent to a (2048,32) @ (32,65536) matmul with the diagonal of each
256x256 archetype zeroed. Sharding: the 65536 output columns are split across
the 8 cores (8192 columns each) — each core then only needs a small slice of
the archetypes plus the full (tiny) weight matrix; the per-core output write
is the roofline.

Precision: fp16 end to end on-device (inputs quantized to fp16 on host, PE
matmul fp16 -> fp32 PSUM, output stored fp16, widened to fp32 on host). This
halves the HBM store traffic (64 MB -> 32 MB/core) vs an fp32 kernel — the
DMA-write roofline — and keeps the PE at full 16-bit rate. Normalized error
vs the fp32 reference is ~5e-4, well inside the 2e-2 gate.

Per-core device layout (all host-side prepped so every DMA is a plain copy):
  wt4   (128, 2048): batch_weights^T replicated into 4 row-groups
                     wt4[32a+k, b] = w[b, k]
  archp (128, 2048): the core's 8192 archetype columns, masked, split into
                     16 chunks of 512; chunk t lives in row-group a = t%4 at
                     quad j = t//4: archp[32a+k, 512j+c] = A[k, 512t+c]
  out   (2048, 8192): the core's output column slice, natural order.

Compute: for each 128-row batch tile, 4 quads x 4 row-tiled fp16 matmuls
(K=32 at tile_position (32a,0)), PSUM split into 1024-col tiles (4 bufs =
all 8 banks) drained by alternating Vector/Scalar casts (fp32 PSUM -> fp16
SBUF; the only two PSUM-capable drain engines on TRN2, both ~95% busy —
the compute-side pace, ~77 us, just under the ~80 us DMA-store pace).

Loads and stores ride the sync HWDGE queue (the gpsimd software-DGE queue
pays ~4 us descriptor generation; the scalar HWDGE queue is no earlier).
Stores use a graded size ramp: 512 KB quads while the 16-engine DMA pipe
fills, 1 MB halves next, then steady-state 2 MB full tiles at the
26.6 GB/s/engine line rate, and 1 MB halves again for the last batch tile
so the final store drains in ~2.4 us. Staging pools are deep (6/4/5 bufs
= ~14 MB) so the copy stream never stalls on staging-buffer recycling
while the DMA works down its ramp-phase backlog — that stall starved the
DMA mid-kernel in earlier revisions (shallow pools stretch the 74 us copy
stream to 80-91 us).

Measured on 8 axon trn2 cores: ~100 us (HW NTFF exec, core 0) on clean
runs vs 186 us for the bit-exact fp32 version of the same pipeline;
occasional runs show a +10-15 us tail when one SDMA engine's final
completion semaphore queues behind cross-core HBM traffic.
"""

import numpy as np

import concourse.tile as tile
from concourse import bacc, mybir
from concourse.bass_utils import run_bass_kernel_spmd

B, K, D = 2048, 32, 256
NCORES = 8
COLS = D * D            # 65536
CPC = COLS // NCORES    # 8192 columns per core
GW = CPC // 4           # 2048 columns per row-group
MT = 128                # batch tile rows (psum partition dim)
NMT = B // MT           # 16 batch tiles
NQ = GW // 512          # 4 quads per batch tile

F16 = mybir.dt.float16
F32 = mybir.dt.float32

_compiled = {}


def _build():
    nc = bacc.Bacc("TRN2", target_bir_lowering=False, debug=False, num_devices=NCORES)
    wt = nc.dram_tensor("wt4", [128, B], F16, kind="ExternalInput").ap()
    # archp stored chunk-major: chunk j is a fully contiguous 128 KB block,
    # so the per-chunk prologue loads run at DMA line rate.
    ar = nc.dram_tensor("archp", [NQ, 128, 512], F16, kind="ExternalInput").ap()
    out = nc.dram_tensor("out", [B, CPC], F16, kind="ExternalOutput").ap()

    with tile.TileContext(nc) as tc:
        with (
            tc.tile_pool(name="wpool", bufs=1) as wpool,
            tc.tile_pool(name="apool", bufs=1) as apool,
            tc.tile_pool(name="pspool", bufs=4, space="PSUM") as pspool,
            tc.tile_pool(name="stpool", bufs=8) as stpool,
        ):
            # Chunked input loads so the first matmuls (needing only wt4 and
            # archp chunk 0) start a couple of microseconds in rather than
            # waiting for the full 1 MB of inputs. All on the sync HWDGE
            # queue: the gpsimd software-DGE queue pays ~4 us of descriptor
            # generation and the scalar HWDGE queue is no earlier, so the
            # simple ordering wins.
            wt_sb = wpool.tile([128, B], F16)
            ar_sb = apool.tile([128, GW], F16)
            nc.sync.dma_start(wt_sb[:, :MT], wt[:, :MT])
            nc.sync.dma_start(ar_sb[:, :512], ar[0])
            nc.sync.dma_start(ar_sb[:, 512:1024], ar[1])
            nc.sync.dma_start(wt_sb[:, MT:], wt[:, MT:])
            for j in range(2, NQ):
                nc.sync.dma_start(ar_sb[:, 512 * j : 512 * (j + 1)], ar[j])

            cnt = 0
            for m in range(NMT):
                # Store-size ramp: small while the DMA pipe fills, full-tile
                # in steady state, small again for the final tile's drain.
                if m < 2:
                    quads_per_store = 1
                elif m < 4 or m == NMT - 1:
                    quads_per_store = 2
                else:
                    quads_per_store = NQ
                for j in range(NQ):
                    if j % quads_per_store == 0:
                        st = stpool.tile(
                            [128, GW * quads_per_store],
                            F16,
                            tag=f"st{quads_per_store}",
                            bufs={1: 6, 2: 4, NQ: 5}[quads_per_store],
                        )
                    for h in range(2):
                        ps = pspool.tile([128, 2 * 512], F32)
                        for i, a in enumerate((2 * h, 2 * h + 1)):
                            nc.tensor.matmul(
                                ps[:, 512 * i : 512 * (i + 1)],
                                wt_sb[32 * a : 32 * (a + 1), MT * m : MT * (m + 1)],
                                ar_sb[32 * a : 32 * (a + 1), 512 * j : 512 * (j + 1)],
                                start=True,
                                stop=True,
                                tile_position=(32 * a, 0),
                            )
                        off = GW * (j % quads_per_store) + 1024 * h
                        dst = st[:, off : off + 1024]
                        # Alternate the PSUM->SBUF drain (fp32 -> fp16 cast)
                        # between the two PSUM-capable engines.
                        if cnt % 2 == 0:
                            nc.vector.tensor_copy(dst, ps[:])
                        else:
                            nc.scalar.copy(dst, ps[:])
                        cnt += 1
                    if (j + 1) % quads_per_store == 0:
                        j0 = j + 1 - quads_per_store
                        nc.sync.dma_start(
                            out[MT * m : MT * (m + 1), GW * j0 : GW * (j + 1)],
                            st[:],
                        )

    nc.compile()
    return nc


def _get_nc():
    if "nc" not in _compiled:
        _compiled["nc"] = _build()
    return _compiled["nc"]


def _prep_inputs(batch_weights: np.ndarray, archs: np.ndarray):
    w = np.asarray(batch_weights, dtype=np.float32)
    A = np.asarray(archs, dtype=np.float32).reshape(K, COLS).copy()
    A[:, :: D + 1] = 0.0  # zero the diagonal of each (D, D) archetype
    A = A.astype(np.float16)

    wt4 = np.ascontiguousarray(np.tile(w.T.astype(np.float16), (4, 1)))  # (128, B)

    in_maps = []
    for c in range(NCORES):
        sl = A[:, CPC * c : CPC * (c + 1)].reshape(K, 4 * NQ, 512)
        archp = np.concatenate(
            [sl[:, a::4, :].reshape(K, NQ, 512) for a in range(4)], axis=0
        )  # (128, NQ, 512); chunk-major DRAM layout is (NQ, 128, 512)
        in_maps.append(
            {"wt4": wt4, "archp": np.ascontiguousarray(archp.transpose(1, 0, 2))}
        )
    return in_maps


def _gather(results) -> np.ndarray:
    outf = np.empty((B, COLS), dtype=np.float32)
    for c in range(NCORES):
        outf[:, CPC * c : CPC * (c + 1)] = results[c]["out"]
    return outf.reshape(B, D, D)


def kernel(batch_weights: np.ndarray, archs: np.ndarray, **run_kwargs) -> np.ndarray:
    nc = _get_nc()
    in_maps = _prep_inputs(batch_weights, archs)
    res = run_bass_kernel_spmd(nc, in_maps, list(range(NCORES)), **run_kwargs)
    if run_kwargs:
        _compiled["last_result"] = res
    return _gather(res.results)


# revision 23
# speedup vs baseline: 1.0379x; 1.0211x over previous
"""Trainium2 Bass kernel for nn_Explainer: out[b] = sum_k w[b,k] * (archs[k] off-diag).

Equivalent to a (2048,32) @ (32,65536) matmul with the diagonal of each
256x256 archetype zeroed. Sharding: the 65536 output columns are split across
the 8 cores (8192 columns each) — each core then only needs a small slice of
the archetypes plus the full (tiny) weight matrix; the per-core output write
is the roofline.

Precision: fp16 end to end on-device (inputs quantized to fp16 on host, PE
matmul fp16 -> fp32 PSUM, output stored fp16, widened to fp32 on host). This
halves the HBM store traffic (64 MB -> 32 MB/core) vs an fp32 kernel — the
DMA-write roofline — and keeps the PE at full 16-bit rate. Normalized error
vs the fp32 reference is ~5e-4, well inside the 2e-2 gate.

Per-core device layout (all host-side prepped so every DMA is a plain copy):
  wt4   (128, 2048): batch_weights^T replicated into 4 row-groups
                     wt4[32a+k, b] = w[b, k]
  archp (128, 2048): the core's 8192 archetype columns, masked, split into
                     16 chunks of 512; chunk t lives in row-group a = t%4 at
                     quad j = t//4: archp[32a+k, 512j+c] = A[k, 512t+c]
  out   (2048, 8192): the core's output column slice, natural order.

Compute: for each 128-row batch tile, 4 quads x 4 row-tiled fp16 matmuls
(K=32 at tile_position (32a,0)), PSUM split into 1024-col tiles (4 bufs =
all 8 banks) drained by alternating Vector/Scalar casts (fp32 PSUM -> fp16
SBUF; the only two PSUM-capable drain engines on TRN2, both ~95% busy —
the compute-side pace, ~77 us, just under the ~80 us DMA-store pace).

Loads and stores ride the sync HWDGE queue (the gpsimd software-DGE queue
pays ~4 us descriptor generation; the scalar HWDGE queue is no earlier).
Stores use a graded size ramp: 512 KB quads while the 16-engine DMA pipe
fills, 1 MB halves next, then steady-state 2 MB full tiles at the
26.6 GB/s/engine line rate, and 1 MB halves again for the last batch tile
so the final store drains in ~2.4 us. Staging pools are deep (6/4/5 bufs
= ~14 MB) so the copy stream never stalls on staging-buffer recycling
while the DMA works down its ramp-phase backlog — that stall starved the
DMA mid-kernel in earlier revisions (shallow pools stretch the 74 us copy
stream to 80-91 us).

Measured on 8 axon trn2 cores: ~100 us (HW NTFF exec, core 0) on clean
runs vs 186 us for the bit-exact fp32 version of the same pipeline;
occasional runs show a +10-15 us tail when one SDMA engine's final
completion semaphore queues behind cross-core HBM traffic.
"""

import numpy as np

import concourse.tile as tile
from concourse import bacc, mybir
from concourse.bass_utils import run_bass_kernel_spmd

B, K, D = 2048, 32, 256
NCORES = 8
COLS = D * D            # 65536
CPC = COLS // NCORES    # 8192 columns per core
GW = CPC // 4           # 2048 columns per row-group
MT = 128                # batch tile rows (psum partition dim)
NMT = B // MT           # 16 batch tiles
NQ = GW // 512          # 4 quads per batch tile

F16 = mybir.dt.float16
F32 = mybir.dt.float32

_compiled = {}


def _build():
    nc = bacc.Bacc("TRN2", target_bir_lowering=False, debug=False, num_devices=NCORES)
    # w^T replicated into 4 row-groups: the matmul's weight operand must
    # start at the same SBUF partition as its fmap (CoreV3 codegen rule),
    # so each 32-row group needs its own copy of w^T.
    wt = nc.dram_tensor("wt4", [128, B], F16, kind="ExternalInput").ap()
    # archp stored chunk-major: chunk j is a fully contiguous 128 KB block,
    # so the per-chunk prologue loads run at DMA line rate.
    ar = nc.dram_tensor("archp", [NQ, 128, 512], F16, kind="ExternalInput").ap()
    out = nc.dram_tensor("out", [B, CPC], F16, kind="ExternalOutput").ap()

    with tile.TileContext(nc) as tc:
        with (
            tc.tile_pool(name="wpool", bufs=1) as wpool,
            tc.tile_pool(name="apool", bufs=1) as apool,
            tc.tile_pool(name="pspool", bufs=4, space="PSUM") as pspool,
            tc.tile_pool(name="stpool", bufs=8) as stpool,
        ):
            # Chunked input loads so the first matmuls (needing only wt4 and
            # archp chunk 0) start a couple of microseconds in rather than
            # waiting for the full 1 MB of inputs. All on the sync HWDGE
            # queue: the gpsimd software-DGE queue pays ~4 us of descriptor
            # generation and the scalar HWDGE queue is no earlier, so the
            # simple ordering wins.
            wt_sb = wpool.tile([128, B], F16)
            ar_sb = apool.tile([128, GW], F16)
            nc.sync.dma_start(wt_sb[:, :MT], wt[:, :MT])
            nc.sync.dma_start(ar_sb[:, :512], ar[0])
            nc.sync.dma_start(ar_sb[:, 512:1024], ar[1])
            nc.sync.dma_start(wt_sb[:, MT:], wt[:, MT:])
            for j in range(2, NQ):
                nc.sync.dma_start(ar_sb[:, 512 * j : 512 * (j + 1)], ar[j])

            cnt = 0
            for m in range(NMT):
                # Store-size ramp: small while the DMA pipe fills, full-tile
                # in steady state, small again for the final tile's drain.
                if m < 2:
                    quads_per_store = 1
                elif m < 4 or m == NMT - 1:
                    quads_per_store = 2
                else:
                    quads_per_store = NQ
                for j in range(NQ):
                    if j % quads_per_store == 0:
                        st = stpool.tile(
                            [128, GW * quads_per_store],
                            F16,
                            tag=f"st{quads_per_store}",
                            bufs={1: 6, 2: 4, NQ: 5}[quads_per_store],
                        )
                    for h in range(2):
                        ps = pspool.tile([128, 2 * 512], F32)
                        for i, a in enumerate((2 * h, 2 * h + 1)):
                            nc.tensor.matmul(
                                ps[:, 512 * i : 512 * (i + 1)],
                                wt_sb[32 * a : 32 * (a + 1), MT * m : MT * (m + 1)],
                                ar_sb[32 * a : 32 * (a + 1), 512 * j : 512 * (j + 1)],
                                start=True,
                                stop=True,
                                tile_position=(32 * a, 0),
                            )
                        off = GW * (j % quads_per_store) + 1024 * h
                        dst = st[:, off : off + 1024]
                        # Alternate the PSUM->SBUF drain (fp32 -> fp16 cast)
                        # between the two PSUM-capable engines.
                        if cnt % 2 == 0:
                            nc.vector.tensor_copy(dst, ps[:])
                        else:
                            nc.scalar.copy(dst, ps[:])
                        cnt += 1
                    if (j + 1) % quads_per_store == 0:
                        j0 = j + 1 - quads_per_store
                        nc.sync.dma_start(
                            out[MT * m : MT * (m + 1), GW * j0 : GW * (j + 1)],
                            st[:],
                        )

    nc.compile()
    return nc


def _get_nc():
    if "nc" not in _compiled:
        _compiled["nc"] = _build()
    return _compiled["nc"]


def _prep_inputs(batch_weights: np.ndarray, archs: np.ndarray):
    w = np.asarray(batch_weights, dtype=np.float32)
    A = np.asarray(archs, dtype=np.float32).reshape(K, COLS).copy()
    A[:, :: D + 1] = 0.0  # zero the diagonal of each (D, D) archetype
    A = A.astype(np.float16)

    wt4 = np.ascontiguousarray(np.tile(w.T.astype(np.float16), (4, 1)))  # (128, B)

    in_maps = []
    for c in range(NCORES):
        sl = A[:, CPC * c : CPC * (c + 1)].reshape(K, 4 * NQ, 512)
        archp = np.concatenate(
            [sl[:, a::4, :].reshape(K, NQ, 512) for a in range(4)], axis=0
        )  # (128, NQ, 512); chunk-major DRAM layout is (NQ, 128, 512)
        in_maps.append(
            {"wt4": wt4, "archp": np.ascontiguousarray(archp.transpose(1, 0, 2))}
        )
    return in_maps


def _gather(results) -> np.ndarray:
    outf = np.empty((B, COLS), dtype=np.float32)
    for c in range(NCORES):
        outf[:, CPC * c : CPC * (c + 1)] = results[c]["out"]
    return outf.reshape(B, D, D)


def kernel(batch_weights: np.ndarray, archs: np.ndarray, **run_kwargs) -> np.ndarray:
    nc = _get_nc()
    in_maps = _prep_inputs(batch_weights, archs)
    res = run_bass_kernel_spmd(nc, in_maps, list(range(NCORES)), **run_kwargs)
    if run_kwargs:
        _compiled["last_result"] = res
    return _gather(res.results)


# revision 24
# speedup vs baseline: 1.0823x; 1.0428x over previous
"""Trainium2 Bass kernel for nn_Explainer: out[b] = sum_k w[b,k] * (archs[k] off-diag).

Equivalent to a (2048,32) @ (32,65536) matmul with the diagonal of each
256x256 archetype zeroed. Sharding: the 65536 output columns are split across
the 8 cores (8192 columns each) — each core then only needs a small slice of
the archetypes plus the full (tiny) weight matrix; the per-core output write
is the roofline.

Precision: fp16 end to end on-device (inputs quantized to fp16 on host, PE
matmul fp16 -> fp32 PSUM, output stored fp16, widened to fp32 on host). This
halves the HBM store traffic (64 MB -> 32 MB/core) vs an fp32 kernel — the
DMA-write roofline — and keeps the PE at full 16-bit rate. Normalized error
vs the fp32 reference is ~5e-4, well inside the 2e-2 gate.

Per-core device layout (all host-side prepped so every DMA is a plain copy):
  wt4   (128, 2048): batch_weights^T replicated into 4 row-groups
                     wt4[32a+k, b] = w[b, k]
  archp (128, 2048): the core's 8192 archetype columns, masked, split into
                     16 chunks of 512; chunk t lives in row-group a = t%4 at
                     quad j = t//4: archp[32a+k, 512j+c] = A[k, 512t+c]
  out   (2048, 8192): the core's output column slice, natural order.

Compute: for each 128-row batch tile, 4 quads x 4 row-tiled fp16 matmuls
(K=32 at tile_position (32a,0)), PSUM split into 1024-col tiles (4 bufs =
all 8 banks) drained by alternating Vector/Scalar casts (fp32 PSUM -> fp16
SBUF; the only two PSUM-capable drain engines on TRN2, both ~95% busy —
the compute-side pace, ~77 us, just under the ~80 us DMA-store pace).

Loads and stores ride the sync HWDGE queue (the gpsimd software-DGE queue
pays ~4 us descriptor generation; the scalar HWDGE queue is no earlier).
Stores use a graded size ramp: 512 KB quads while the 16-engine DMA pipe
fills, 1 MB halves next, then steady-state 2 MB full tiles at the
26.6 GB/s/engine line rate, and 1 MB halves again for the last batch tile
so the final store drains in ~2.4 us. Staging pools are deep (6/4/5 bufs
= ~14 MB) so the copy stream never stalls on staging-buffer recycling
while the DMA works down its ramp-phase backlog — that stall starved the
DMA mid-kernel in earlier revisions (shallow pools stretch the 74 us copy
stream to 80-91 us).

Measured on 8 axon trn2 cores: ~100 us (HW NTFF exec, core 0) on clean
runs vs 186 us for the bit-exact fp32 version of the same pipeline;
occasional runs show a +10-15 us tail when one SDMA engine's final
completion semaphore queues behind cross-core HBM traffic.
"""

import numpy as np

import concourse.tile as tile
from concourse import bacc, mybir
from concourse.bass_utils import run_bass_kernel_spmd

B, K, D = 2048, 32, 256
NCORES = 8
COLS = D * D            # 65536
CPC = COLS // NCORES    # 8192 columns per core
GW = CPC // 4           # 2048 columns per row-group
MT = 128                # batch tile rows (psum partition dim)
NMT = B // MT           # 16 batch tiles
NQ = GW // 512          # 4 quads per batch tile

F16 = mybir.dt.float16
F32 = mybir.dt.float32

_compiled = {}


def _build():
    nc = bacc.Bacc("TRN2", target_bir_lowering=False, debug=False, num_devices=NCORES)
    # w^T replicated into 4 row-groups: the matmul's weight operand must
    # start at the same SBUF partition as its fmap (CoreV3 codegen rule),
    # so each 32-row group needs its own copy of w^T.
    wt = nc.dram_tensor("wt4", [128, B], F16, kind="ExternalInput").ap()
    # archp stored chunk-major: chunk j is a fully contiguous 128 KB block,
    # so the per-chunk prologue loads run at DMA line rate.
    ar = nc.dram_tensor("archp", [NQ, 128, 512], F16, kind="ExternalInput").ap()
    out = nc.dram_tensor("out", [B, CPC], F16, kind="ExternalOutput").ap()

    with tile.TileContext(nc) as tc:
        with (
            tc.tile_pool(name="wpool", bufs=1) as wpool,
            tc.tile_pool(name="apool", bufs=1) as apool,
            tc.tile_pool(name="pspool", bufs=4, space="PSUM") as pspool,
            tc.tile_pool(name="stpool", bufs=8) as stpool,
        ):
            # Chunked input loads so the first matmuls (needing only wt4 and
            # archp chunk 0) start a couple of microseconds in rather than
            # waiting for the full 1 MB of inputs. All on the sync HWDGE
            # queue: the gpsimd software-DGE queue pays ~4 us of descriptor
            # generation and the scalar HWDGE queue is no earlier, so the
            # simple ordering wins.
            wt_sb = wpool.tile([128, B], F16)
            ar_sb = apool.tile([128, GW], F16)
            nc.sync.dma_start(wt_sb[:, :MT], wt[:, :MT])
            nc.sync.dma_start(ar_sb[:, :512], ar[0])
            nc.sync.dma_start(ar_sb[:, 512:1024], ar[1])
            nc.sync.dma_start(wt_sb[:, MT:], wt[:, MT:])
            for j in range(2, NQ):
                nc.sync.dma_start(ar_sb[:, 512 * j : 512 * (j + 1)], ar[j])

            cnt = 0
            for m in range(NMT):
                # Store-size ramp: small while the DMA pipe fills, full-tile
                # in steady state, small again for the final tile's drain.
                if m < 2 or m == NMT - 1:
                    quads_per_store = 1
                elif m < 4:
                    quads_per_store = 2
                else:
                    quads_per_store = NQ
                for j in range(NQ):
                    if j % quads_per_store == 0:
                        st = stpool.tile(
                            [128, GW * quads_per_store],
                            F16,
                            tag=f"st{quads_per_store}",
                            bufs={1: 6, 2: 4, NQ: 5}[quads_per_store],
                        )
                    for h in range(2):
                        ps = pspool.tile([128, 2 * 512], F32)
                        for i, a in enumerate((2 * h, 2 * h + 1)):
                            nc.tensor.matmul(
                                ps[:, 512 * i : 512 * (i + 1)],
                                wt_sb[32 * a : 32 * (a + 1), MT * m : MT * (m + 1)],
                                ar_sb[32 * a : 32 * (a + 1), 512 * j : 512 * (j + 1)],
                                start=True,
                                stop=True,
                                tile_position=(32 * a, 0),
                            )
                        off = GW * (j % quads_per_store) + 1024 * h
                        dst = st[:, off : off + 1024]
                        # Alternate the PSUM->SBUF drain (fp32 -> fp16 cast)
                        # between the two PSUM-capable engines.
                        if cnt % 2 == 0:
                            nc.vector.tensor_copy(dst, ps[:])
                        else:
                            nc.scalar.copy(dst, ps[:])
                        cnt += 1
                    if (j + 1) % quads_per_store == 0:
                        j0 = j + 1 - quads_per_store
                        nc.sync.dma_start(
                            out[MT * m : MT * (m + 1), GW * j0 : GW * (j + 1)],
                            st[:],
                        )

    nc.compile()
    return nc


def _get_nc():
    if "nc" not in _compiled:
        _compiled["nc"] = _build()
    return _compiled["nc"]


def _prep_inputs(batch_weights: np.ndarray, archs: np.ndarray):
    w = np.asarray(batch_weights, dtype=np.float32)
    A = np.asarray(archs, dtype=np.float32).reshape(K, COLS).copy()
    A[:, :: D + 1] = 0.0  # zero the diagonal of each (D, D) archetype
    A = A.astype(np.float16)

    wt4 = np.ascontiguousarray(np.tile(w.T.astype(np.float16), (4, 1)))  # (128, B)

    in_maps = []
    for c in range(NCORES):
        sl = A[:, CPC * c : CPC * (c + 1)].reshape(K, 4 * NQ, 512)
        archp = np.concatenate(
            [sl[:, a::4, :].reshape(K, NQ, 512) for a in range(4)], axis=0
        )  # (128, NQ, 512); chunk-major DRAM layout is (NQ, 128, 512)
        in_maps.append(
            {"wt4": wt4, "archp": np.ascontiguousarray(archp.transpose(1, 0, 2))}
        )
    return in_maps


def _gather(results) -> np.ndarray:
    outf = np.empty((B, COLS), dtype=np.float32)
    for c in range(NCORES):
        outf[:, CPC * c : CPC * (c + 1)] = results[c]["out"]
    return outf.reshape(B, D, D)


def kernel(batch_weights: np.ndarray, archs: np.ndarray, **run_kwargs) -> np.ndarray:
    nc = _get_nc()
    in_maps = _prep_inputs(batch_weights, archs)
    res = run_bass_kernel_spmd(nc, in_maps, list(range(NCORES)), **run_kwargs)
    if run_kwargs:
        _compiled["last_result"] = res
    return _gather(res.results)


# revision 26
# speedup vs baseline: 1.1727x; 1.0835x over previous
"""Trainium2 Bass kernel for nn_Explainer: out[b] = sum_k w[b,k] * (archs[k] off-diag).

Equivalent to a (2048,32) @ (32,65536) matmul with the diagonal of each
256x256 archetype zeroed. Sharding: the 65536 output columns are split across
the 8 cores (8192 columns each) — each core then only needs a small slice of
the archetypes plus the full (tiny) weight matrix; the per-core output write
is the roofline.

Precision: fp16 end to end on-device (inputs quantized to fp16 on host, PE
matmul fp16 -> fp32 PSUM, output stored fp16, widened to fp32 on host). This
halves the HBM store traffic (64 MB -> 32 MB/core) vs an fp32 kernel — the
DMA-write roofline — and keeps the PE at full 16-bit rate. Normalized error
vs the fp32 reference is ~5e-4, well inside the 2e-2 gate.

Per-core device layout (all host-side prepped so every DMA is a plain copy):
  wt4   (128, 2048): batch_weights^T replicated into 4 row-groups
                     wt4[32a+k, b] = w[b, k]
  archp (128, 2048): the core's 8192 archetype columns, masked, split into
                     16 chunks of 512; chunk t lives in row-group a = t%4 at
                     quad j = t//4: archp[32a+k, 512j+c] = A[k, 512t+c]
  out   (2048, 8192): the core's output column slice, natural order.

Compute: for each 128-row batch tile, 4 quads x 4 row-tiled fp16 matmuls
(K=32 at tile_position (32a,0)), PSUM split into 1024-col tiles (4 bufs =
all 8 banks) drained by alternating Vector/Scalar casts (fp32 PSUM -> fp16
SBUF; the only two PSUM-capable drain engines on TRN2, both ~95% busy —
the compute-side pace, ~77 us, just under the ~80 us DMA-store pace).

Loads and stores ride the sync HWDGE queue (the gpsimd software-DGE queue
pays ~4 us descriptor generation; the scalar HWDGE queue is no earlier).
Stores use a graded size ramp: 512 KB quads while the 16-engine DMA pipe
fills, 1 MB halves next, then steady-state 2 MB full tiles at the
26.6 GB/s/engine line rate, and 512 KB quads again for the last batch
tile so the final store and its completion semaphores clear the queue
~1 us sooner. Staging pools are deep (6/4/5 bufs
= ~14 MB) so the copy stream never stalls on staging-buffer recycling
while the DMA works down its ramp-phase backlog — that stall starved the
DMA mid-kernel in earlier revisions (shallow pools stretch the 74 us copy
stream to 80-91 us).

Measured on 8 axon trn2 cores: ~100 us (HW NTFF exec, core 0) on clean
runs vs 186 us for the bit-exact fp32 version of the same pipeline;
occasional runs show a +10-15 us tail when one SDMA engine's final
completion semaphore queues behind cross-core HBM traffic.
"""

import numpy as np

import concourse.tile as tile
from concourse import bacc, mybir
from concourse.bass_utils import run_bass_kernel_spmd

B, K, D = 2048, 32, 256
NCORES = 8
COLS = D * D            # 65536
CPC = COLS // NCORES    # 8192 columns per core
GW = CPC // 4           # 2048 columns per row-group
MT = 128                # batch tile rows (psum partition dim)
NMT = B // MT           # 16 batch tiles
NQ = GW // 512          # 4 quads per batch tile

F16 = mybir.dt.float16
F32 = mybir.dt.float32

_compiled = {}


def _build():
    nc = bacc.Bacc("TRN2", target_bir_lowering=False, debug=False, num_devices=NCORES)
    # w^T replicated into 4 row-groups: the matmul's weight operand must
    # start at the same SBUF partition as its fmap (CoreV3 codegen rule),
    # so each 32-row group needs its own copy of w^T.
    wt = nc.dram_tensor("wt4", [128, B], F16, kind="ExternalInput").ap()
    # archp stored chunk-major: chunk j is a fully contiguous 128 KB block,
    # so the per-chunk prologue loads run at DMA line rate.
    ar = nc.dram_tensor("archp", [NQ, 128, 512], F16, kind="ExternalInput").ap()
    out = nc.dram_tensor("out", [B, CPC], F16, kind="ExternalOutput").ap()

    with tile.TileContext(nc) as tc:
        with (
            tc.tile_pool(name="wpool", bufs=1) as wpool,
            tc.tile_pool(name="apool", bufs=1) as apool,
            tc.tile_pool(name="pspool", bufs=4, space="PSUM") as pspool,
            tc.tile_pool(name="stpool", bufs=8) as stpool,
        ):
            # Chunked input loads so the first matmuls (needing only wt4 and
            # archp chunk 0) start a couple of microseconds in rather than
            # waiting for the full 1 MB of inputs. All on the sync HWDGE
            # queue: the gpsimd software-DGE queue pays ~4 us of descriptor
            # generation and the scalar HWDGE queue is no earlier, so the
            # simple ordering wins.
            wt_sb = wpool.tile([128, B], F16)
            ar_sb = apool.tile([128, GW], F16)
            nc.sync.dma_start(wt_sb[:, :MT], wt[:, :MT])
            nc.sync.dma_start(ar_sb[:, :512], ar[0])
            nc.sync.dma_start(ar_sb[:, 512:1024], ar[1])
            nc.sync.dma_start(wt_sb[:, MT:], wt[:, MT:])
            for j in range(2, NQ):
                nc.sync.dma_start(ar_sb[:, 512 * j : 512 * (j + 1)], ar[j])

            cnt = 0
            for m in range(NMT):
                # Store-size ramp: small while the DMA pipe fills, full-tile
                # in steady state, small again for the final tile's drain.
                if m < 2 or m == NMT - 1:
                    quads_per_store = 1
                elif m < 4:
                    quads_per_store = 2
                else:
                    quads_per_store = NQ
                for j in range(NQ):
                    if j % quads_per_store == 0:
                        st = stpool.tile(
                            [128, GW * quads_per_store],
                            F16,
                            tag=f"st{quads_per_store}",
                            bufs={1: 6, 2: 4, NQ: 5}[quads_per_store],
                        )
                    for h in range(2):
                        ps = pspool.tile([128, 2 * 512], F32)
                        for i, a in enumerate((2 * h, 2 * h + 1)):
                            nc.tensor.matmul(
                                ps[:, 512 * i : 512 * (i + 1)],
                                wt_sb[32 * a : 32 * (a + 1), MT * m : MT * (m + 1)],
                                ar_sb[32 * a : 32 * (a + 1), 512 * j : 512 * (j + 1)],
                                start=True,
                                stop=True,
                                tile_position=(32 * a, 0),
                            )
                        off = GW * (j % quads_per_store) + 1024 * h
                        dst = st[:, off : off + 1024]
                        # Alternate the PSUM->SBUF drain (fp32 -> fp16 cast)
                        # between the two PSUM-capable engines.
                        if cnt % 2 == 0:
                            nc.vector.tensor_copy(dst, ps[:])
                        else:
                            nc.scalar.copy(dst, ps[:])
                        cnt += 1
                        if m == 0 and j == 0:
                            # Fire the very first output bytes after ONE
                            # copy so the store pipe starts ~0.7 us sooner.
                            nc.sync.dma_start(
                                out[:MT, 1024 * h : 1024 * (h + 1)],
                                st[:, 1024 * h : 1024 * (h + 1)],
                            )
                    if (j + 1) % quads_per_store == 0 and not (m == 0 and j == 0):
                        j0 = j + 1 - quads_per_store
                        nc.sync.dma_start(
                            out[MT * m : MT * (m + 1), GW * j0 : GW * (j + 1)],
                            st[:],
                        )

    nc.compile()
    return nc


def _get_nc():
    if "nc" not in _compiled:
        _compiled["nc"] = _build()
    return _compiled["nc"]


def _prep_inputs(batch_weights: np.ndarray, archs: np.ndarray):
    w = np.asarray(batch_weights, dtype=np.float32)
    A = np.asarray(archs, dtype=np.float32).reshape(K, COLS).copy()
    A[:, :: D + 1] = 0.0  # zero the diagonal of each (D, D) archetype
    A = A.astype(np.float16)

    wt4 = np.ascontiguousarray(np.tile(w.T.astype(np.float16), (4, 1)))  # (128, B)

    in_maps = []
    for c in range(NCORES):
        sl = A[:, CPC * c : CPC * (c + 1)].reshape(K, 4 * NQ, 512)
        archp = np.concatenate(
            [sl[:, a::4, :].reshape(K, NQ, 512) for a in range(4)], axis=0
        )  # (128, NQ, 512); chunk-major DRAM layout is (NQ, 128, 512)
        in_maps.append(
            {"wt4": wt4, "archp": np.ascontiguousarray(archp.transpose(1, 0, 2))}
        )
    return in_maps


def _gather(results) -> np.ndarray:
    outf = np.empty((B, COLS), dtype=np.float32)
    for c in range(NCORES):
        outf[:, CPC * c : CPC * (c + 1)] = results[c]["out"]
    return outf.reshape(B, D, D)


def kernel(batch_weights: np.ndarray, archs: np.ndarray, **run_kwargs) -> np.ndarray:
    nc = _get_nc()
    in_maps = _prep_inputs(batch_weights, archs)
    res = run_bass_kernel_spmd(nc, in_maps, list(range(NCORES)), **run_kwargs)
    if run_kwargs:
        _compiled["last_result"] = res
    return _gather(res.results)
